# revision 1
# baseline (speedup 1.0000x reference)
"""Trainium2 Bass kernel for nn_Encoder (sliding-window MLP + synaptic conv).

Computation (per timestep t of T_data):
  syn_e[t] = sum(S_e[t, :]);  syn_i[t] = sum(S_i[t, :])
  syn_out[t, s] = sum_k e_kern[s, k] * syn_e[t-k] + i_kern[s, k] * syn_i[t-k]
  Vw[t, :] = V[t-199 : t+200]   (zero padded)
  h = lrelu(Vw @ W1.T + b1); h = lrelu(h @ W2.T + b2); h = lrelu(h @ W3.T + b3)
  out[t, :] = tanh(h @ W4.T + b4 + syn_out[t, :])

Strategy: data-parallel over T across 8 NeuronCores, each core gets its
T/8 slice plus a 199-row halo of S_e/S_i and a 398-elem halo of V (host
zero-pads the edges).  On each core:
  - S_e/S_i row-sums via VectorE free-axis reduce (fp32), PE-transposed and
    cast to bf16, stored contiguously to a DRAM scratch vector.
  - All matmuls in bf16 (fp32 PSUM accumulate).  Sliding windows of V and
    syn_e/syn_i are materialized as matmul operands directly by overlapping
    ("Hankel") DMA access patterns from DRAM: partition step 1, free step 1.
    One wide Hankel tile per block serves all K-chunks as column slices.
  - Layers 1-3 keep hid on PSUM partitions; layer 4 + conv keep the
    [sub, t] orientation (weights stationary) so the output store is a
    contiguous [sub, nt] tile; the host transposes the gathered output.
    b4 is added via a K=1 matmul (b4 stationary, ones streaming).
  - lrelu/tanh run on ScalarE (Lrelu alpha=0.01, Tanh); both live in the
    same ACT LUT table set so only one table load is emitted.
"""

import os
from contextlib import ExitStack

import ml_dtypes
import numpy as np

import concourse.bass as bass
import concourse.mybir as mybir
import concourse.tile as tile
from concourse import bacc
from concourse.bass_utils import run_bass_kernel_spmd
from concourse.masks import make_identity
from concourse.tile_rust import add_dep_helper

BF16 = ml_dtypes.bfloat16
FP32 = mybir.dt.float32
BF = mybir.dt.bfloat16

T_NO = 200
WIN = 2 * T_NO - 1  # 399
N_CORES = 8
BLK = 512  # timesteps per block (one PSUM bank of fp32)

LAST = {}  # exec_time_ns / trace info from the most recent run (for test harness)


def _ceil_div(a, b):
    return -(-a // b)


def _chunks(total, step=128):
    out = []
    o = 0
    while o < total:
        out.append((o, min(step, total - o)))
        o += step
    return out


def _build(T_PAD, SE_ROWS, E_COLS, I_COLS, HID, SUB):
    """Build the per-core Bass program (identical on all 8 cores)."""
    R_TILES = _ceil_div(SE_ROWS, 128)
    # +128 margin: the wide hankel DMA loads a full [128, nt+W] rectangle
    # whose unused corner reads past the logical end
    SCR_LEN = R_TILES * 128 + 128
    V_LEN = T_PAD + WIN - 1 + 128
    NB = _ceil_div(T_PAD, BLK)

    m_hid = _chunks(HID)  # M chunks of hid (PSUM partitions L1-3)
    k_win = _chunks(WIN)  # K chunks of the V window
    k_hid = _chunks(HID)  # K chunks of hid (L2-4 contraction)
    k_syn = _chunks(T_NO)  # K chunks of the conv kernel length
    VH_W = 128 * (len(k_win) - 1)  # extra hankel cols so K-chunks are slices
    SY_W = 128 * (len(k_syn) - 1)

    nc = bacc.Bacc(
        "TRN2", target_bir_lowering=False, debug=False, num_devices=N_CORES
    )

    se_h = nc.dram_tensor("se", [SE_ROWS, E_COLS], FP32, kind="ExternalInput")
    si_h = nc.dram_tensor("si", [SE_ROWS, I_COLS], FP32, kind="ExternalInput")
    v_h = nc.dram_tensor("v", [V_LEN], BF, kind="ExternalInput")
    n_kw, n_kh, n_ks = len(k_win), len(k_hid), len(k_syn)
    w1t_h = nc.dram_tensor("w1t", [128, n_kw * HID], BF, kind="ExternalInput")
    w2t_h = nc.dram_tensor("w2t", [128, n_kh * HID], BF, kind="ExternalInput")
    w3t_h = nc.dram_tensor("w3t", [128, n_kh * HID], BF, kind="ExternalInput")
    b1_h = nc.dram_tensor("bpk", [128, 3 * n_kh], FP32, kind="ExternalInput")
    b4_h = nc.dram_tensor("b4", [SUB], BF, kind="ExternalInput")
    ekm_h = nc.dram_tensor("spk", [128, (n_kh + 2 * n_ks) * SUB], BF,
                           kind="ExternalInput")
    out_h = nc.dram_tensor("out", [SUB, T_PAD], FP32, kind="ExternalOutput")

    sse_h = nc.dram_tensor("sse_scratch", [SCR_LEN], BF)
    ssi_h = nc.dram_tensor("ssi_scratch", [SCR_LEN], BF)

    with tile.TileContext(nc) as tc, ExitStack() as ctx:
        cpool = ctx.enter_context(tc.tile_pool(name="consts", bufs=1))
        sepool = ctx.enter_context(tc.tile_pool(name="sein", bufs=10))
        accpool = ctx.enter_context(tc.tile_pool(name="acc", bufs=1))
        stpool = ctx.enter_context(tc.tile_pool(name="store", bufs=2))
        hkpool = ctx.enter_context(tc.tile_pool(name="hankel", bufs=3))
        hpool = ctx.enter_context(tc.tile_pool(name="acts", bufs=2))
        opool = ctx.enter_context(tc.tile_pool(name="outs", bufs=3))
        psmm = ctx.enter_context(tc.tile_pool(name="psmm", bufs=5, space="PSUM"))
        ps4p = ctx.enter_context(tc.tile_pool(name="ps4p", bufs=2, space="PSUM"))
        ptrp = ctx.enter_context(tc.tile_pool(name="ptrp", bufs=1, space="PSUM"))

        # ---- constants to SBUF (host-packed: one wide DMA per group) ----
        # w1t/w2t/w3t arrive packed as [128, nchunks*HID]: K-chunk kc of the
        # pre-transposed weight lives at columns [HID*kc, HID*(kc+1))
        def packed_w(dram, nm, ncols):
            t = cpool.tile([128, ncols], BF, name=nm, tag=nm)
            nc.sync.dma_start(out=t[:, :], in_=dram[:, :])
            return t

        w1t_pk = packed_w(w1t_h, "w1t", len(k_win) * HID)
        w2t_pk = packed_w(w2t_h, "w2t", len(k_hid) * HID)
        w3t_pk = packed_w(w3t_h, "w3t", len(k_hid) * HID)
        w1t_sb = [w1t_pk[:, HID * kc : HID * (kc + 1)] for kc in range(len(k_win))]
        w2t_sb = [w2t_pk[:, HID * kc : HID * (kc + 1)] for kc in range(len(k_hid))]
        w3t_sb = [w3t_pk[:, HID * kc : HID * (kc + 1)] for kc in range(len(k_hid))]
        # small pack: [w4t chunks | ek chunks | ik chunks] as [128, (4+2+2)*SUB]
        sp = cpool.tile([128, (len(k_hid) + 2 * len(k_syn)) * SUB], BF,
                        name="smallpk", tag="smallpk")
        nc.sync.dma_start(out=sp[:, :], in_=ekm_h[:, :])
        w4t_sb = [sp[:, SUB * kc : SUB * (kc + 1)] for kc in range(len(k_hid))]
        o1 = len(k_hid)
        ek_sb = [sp[:, SUB * (o1 + j) : SUB * (o1 + j + 1)] for j in range(len(k_syn))]
        o2 = o1 + len(k_syn)
        ik_sb = [sp[:, SUB * (o2 + j) : SUB * (o2 + j + 1)] for j in range(len(k_syn))]
        # biases packed [128, 3*nchunks] f32
        bp = cpool.tile([128, 3 * len(m_hid)], FP32, name="biaspk", tag="biaspk")
        nc.sync.dma_start(out=bp[:, :], in_=b1_h[:, :])
        bias_sb = {f"b{li + 1}": bp[:, li * len(m_hid) : (li + 1) * len(m_hid)]
                   for li in range(3)}

        b4_sb = cpool.tile([1, SUB], BF, name="b4sb", tag="b4sb")
        b4_dma = nc.sync.dma_start(
            out=b4_sb[0:1, :], in_=bass.AP(b4_h, 0, [[0, 1], [1, SUB]]))
        ones_sb = cpool.tile([1, BLK], BF, name="ones", tag="ones")
        nc.vector.memset(ones_sb[0:1, :], 1.0)
        ident = cpool.tile([128, 128], FP32, name="ident", tag="ident")
        make_identity(nc, ident[:, :])

        # ---- reduction accumulators ----
        se_acc = accpool.tile([128, R_TILES], FP32, name="se_acc", tag="se_acc")
        si_acc = accpool.tile([128, R_TILES], FP32, name="si_acc", tag="si_acc")
        if SE_ROWS % 128 != 0:
            # rows past SE_ROWS in the last reduce tile are never written by
            # the reduce; zero them so the scratch tail holds no garbage
            nc.vector.memset(se_acc[:, R_TILES - 1 : R_TILES], 0.0)
            nc.vector.memset(si_acc[:, R_TILES - 1 : R_TILES], 0.0)

        reduced = 0  # reduce tiles emitted so far
        stored = 0  # scratch columns stored so far

        def emit_reduce(i):
            r0 = 128 * i
            nr = min(128, SE_ROWS - r0)
            se_t = sepool.tile([128, E_COLS], FP32, name="se_t", tag="se_t")
            first = nc.gpsimd.dma_start(out=se_t[:nr, :], in_=se_h[r0 : r0 + nr, :])
            nc.vector.reduce_sum(se_acc[:nr, i : i + 1], se_t[:nr, :],
                                 axis=mybir.AxisListType.X)
            si_t = sepool.tile([128, I_COLS], FP32, name="si_t", tag="si_t")
            nc.gpsimd.dma_start(out=si_t[:nr, :], in_=si_h[r0 : r0 + nr, :])
            # S_i row-sums on ScalarE (activation accumulate) to keep the
            # VectorE reduce stream ahead of the DMA stream
            dump = sepool.tile([128, I_COLS], BF, name="si_dump", tag="si_dump",
                               bufs=2)
            nc.scalar.activation(dump[:nr, :], si_t[:nr, :],
                                 mybir.ActivationFunctionType.Identity,
                                 accum_out=si_acc[:nr, i : i + 1])
            return first

        def emit_store(a, b):
            # PE-transpose fp32 accumulator cols [a,b) to [w,128], cast to
            # bf16 on ScalarE, store contiguously to the scratch vector
            w = b - a
            for nm, acc, scr in (("se", se_acc, sse_h), ("si", si_acc, ssi_h)):
                tr_t = ptrp.tile([16, 128], FP32, name=f"{nm}tr", tag="tr")
                nc.tensor.transpose(tr_t[:w, :], acc[:, a:b], ident[:, :])
                st_t = stpool.tile([16, 128], BF, name=f"{nm}st", tag=f"{nm}st")
                nc.vector.tensor_copy(st_t[:w, :], tr_t[:w, :])
                nc.scalar.dma_start(
                    out=bass.AP(scr, 128 * a, [[128, w], [1, 128]]),
                    in_=st_t[:w, :],
                )

        # ---- main loop over timestep block pairs ----
        # Blocks are processed in pairs, interleaved at layer granularity:
        # L1(a) L1(b) L2(a) L2(b) ... so the PE never waits for the ScalarE
        # PSUM evacuation of the previous layer (it runs under the other
        # block's matmuls) and stays HAM-warm.  One wide Hankel tile per
        # pair serves both blocks and all K-chunks as column slices.
        # The synaptic-conv contribution is fully decoupled from the MLP:
        # the feed-forward part (W4 h3 + b4) is evacuated to an SBUF "ff"
        # buffer with the pipeline, and the conv matmuls for pair p execute
        # two pairs later -- by then the bulk reduce stream has long
        # produced their scratch operands, so they never stall the PE.
        pairs = [tuple(b for b in (i, i + 1) if b < NB) for i in range(0, NB, 2)]
        NP = len(pairs)
        pair_blks = []
        needs = []
        for pi, pair in enumerate(pairs):
            blks = []
            off = 0
            for b in pair:
                nt = min(BLK, T_PAD - BLK * b)
                blks.append((BLK * b, nt, off))
                off += nt
            pair_blks.append((blks, off))
            lt0, lnt, _ = blks[-1]
            needs.append(R_TILES if pi == NP - 1 else
                         min(R_TILES, _ceil_div(lt0 + lnt + T_NO - 1, 128)))

        synh_tiles = {}
        ff_tiles = {}

        def emit_synh(pi):
            t0p = BLK * pairs[pi][0]
            tot = pair_blks[pi][1]
            synh = {}
            for nm, scr in (("se", sse_h), ("si", ssi_h)):
                t = hkpool.tile([128, 2 * BLK + SY_W], BF, name=f"{nm}h",
                                tag=f"{nm}h")
                nc.sync.dma_start(
                    out=t[:, : tot + SY_W],
                    in_=bass.AP(scr, t0p, [[1, 128], [1, tot + SY_W]]),
                )
                synh[nm] = t
            synh_tiles[pi] = synh

        def emit_stores_until(tgt):
            nonlocal stored
            while stored < tgt:
                emit_store(stored, min(tgt, stored + 16))
                stored = min(tgt, stored + 16)

        def emit_reduces_until(tgt, gate=None):
            nonlocal reduced
            while reduced < tgt:
                first = emit_reduce(reduced)
                if reduced == 0 and gate is not None:
                    # let the startup-critical weight/hankel loads win the
                    # fabric before the bulk stream starts
                    add_dep_helper(first.ins, gate.ins, sync=True,
                                   reason="gate bulk stream on startup loads")
                reduced += 1

        def emit_conv(pi):
            # conv matmuls + ff add + tanh + store for a pair whose ff and
            # scratch hankel operands were produced pairs ago
            blks, tot = pair_blks[pi]
            synh = synh_tiles.pop(pi)
            ffs = ff_tiles.pop(pi)
            for si_, (bt0, nt, coff) in enumerate(blks):
                ps4 = ps4p.tile([SUB, BLK], FP32, name="ps4c", tag="ps4")
                first = True
                for nm, k_sb in (("se", ek_sb), ("si", ik_sb)):
                    for j, (o, pk) in enumerate(k_syn):
                        last = nm == "si" and j == len(k_syn) - 1
                        nc.tensor.matmul(
                            ps4[:, :nt], k_sb[j][:pk, :],
                            synh[nm][:pk, coff + 128 * j : coff + 128 * j + nt],
                            start=first, stop=last,
                        )
                        first = False
                sum_sb = opool.tile([SUB, BLK], FP32, name="sum_sb", tag="sum_sb")
                nc.vector.tensor_add(sum_sb[:, :nt], ps4[:, :nt], ffs[si_][:, :nt])
                out_sb = opool.tile([SUB, BLK], FP32, name="out_sb", tag="out_sb")
                nc.scalar.activation(out_sb[:, :nt], sum_sb[:, :nt],
                                     mybir.ActivationFunctionType.Tanh)
                nc.sync.dma_start(out=out_h[:, bt0 : bt0 + nt], in_=out_sb[:, :nt])

        for pi, pair in enumerate(pairs):
            blks, tot = pair_blks[pi]
            t0p = BLK * pair[0]

            vh = hkpool.tile([128, 2 * BLK + VH_W], BF, name="vh", tag="vh")
            vh_dma = nc.sync.dma_start(
                out=vh[:, : tot + VH_W],
                in_=bass.AP(v_h, t0p, [[1, 128], [1, tot + VH_W]]),
            )
            if pi == 0:
                emit_reduces_until(needs[0], gate=vh_dma)

            # layers 1..3 (hid on PSUM partitions), block-pair interleaved
            h_prev = {}
            for lidx, (k_list, w_sb, bias_t) in enumerate((
                (k_win, w1t_sb, bias_sb["b1"]),
                (k_hid, w2t_sb, bias_sb["b2"]),
                (k_hid, w3t_sb, bias_sb["b3"]),
            )):
                for si_, (bt0, nt, coff) in enumerate(blks):
                    outs = []
                    for mc, (mo, nm_) in enumerate(m_hid):
                        ps = psmm.tile([128, BLK], FP32, name="ps", tag="ps")
                        for kc, (ko, pk) in enumerate(k_list):
                            if lidx == 0:
                                rhs = vh[:pk, coff + 128 * kc : coff + 128 * kc + nt]
                            else:
                                rhs = h_prev[si_][kc][:pk, :nt]
                            nc.tensor.matmul(
                                ps[:nm_, :nt],
                                w_sb[kc][:pk, mo : mo + nm_],
                                rhs,
                                start=(kc == 0),
                                stop=(kc == len(k_list) - 1),
                            )
                        h_t = hpool.tile([128, BLK], BF, name=f"h{lidx}_{mc}",
                                         tag=f"h{lidx}_{mc}")
                        nc.scalar.activation(
                            h_t[:nm_, :nt], ps[:nm_, :nt],
                            mybir.ActivationFunctionType.Lrelu,
                            bias=bias_t[:nm_, mc : mc + 1], alpha=0.01,
                        )
                        outs.append(h_t)
                    h_prev[si_] = outs

            # feed-forward part of layer 4: ff = W4 h3 + b4 -> SBUF
            ffs = []
            for si_, (bt0, nt, coff) in enumerate(blks):
                h3 = h_prev[si_]
                ps4 = ps4p.tile([SUB, BLK], FP32, name="ps4", tag="ps4")
                nc.tensor.matmul(ps4[:, :nt], b4_sb[0:1, :], ones_sb[0:1, :nt],
                                 start=True, stop=False)
                for kc, (ko, pk) in enumerate(k_hid):
                    nc.tensor.matmul(ps4[:, :nt], w4t_sb[kc][:pk, :],
                                     h3[kc][:pk, :nt], start=False,
                                     stop=(kc == len(k_hid) - 1))
                ff_t = opool.tile([SUB, BLK], FP32, name="ff_sb", tag="ff_sb",
                                  bufs=6)
                nc.scalar.activation(ff_t[:, :nt], ps4[:, :nt],
                                     mybir.ActivationFunctionType.Copy)
                ffs.append(ff_t)
            ff_tiles[pi] = ffs

            # this pair's scratch stores + hankel reloads (consumed by the
            # conv two pairs later); transposes land in the PE queue here
            emit_stores_until(needs[pi])
            emit_synh(pi)

            if pi >= 2:
                emit_conv(pi - 2)

            # bulk reduce loads one pair ahead
            emit_reduces_until(needs[min(pi + 1, NP - 1)])

        for pi in range(max(0, NP - 2), NP):
            emit_conv(pi)

    nc.compile()
    return nc


def kernel(V, S_e, S_i, W1, b1, W2, b2, W3, b3, W4, b4, W_syn, Tau_syn, Delta_syn):
    V = np.asarray(V, np.float32)
    S_e = np.ascontiguousarray(np.asarray(S_e, np.float32))
    S_i = np.ascontiguousarray(np.asarray(S_i, np.float32))
    T = V.shape[0]
    assert T % N_CORES == 0
    T_LOC = T // N_CORES
    T_PAD = _ceil_div(T_LOC, 128) * 128
    SE_ROWS = T_NO - 1 + T_LOC
    V_LEN = T_PAD + WIN - 1 + 128
    HID = W1.shape[0]
    SUB = W4.shape[0]

    # ---- tiny host-side prep (layout/dtype only + 20x200 conv kernels) ----
    W_syn = np.asarray(W_syn, np.float32)
    Tau_syn = np.asarray(Tau_syn, np.float32)
    Delta_syn = np.asarray(Delta_syn, np.float32)
    t_raw = np.arange(T_NO, dtype=np.float32)[None, :]
    t_e = np.maximum(t_raw - Delta_syn[:, 0:1], 0.0)
    t_i = np.maximum(t_raw - Delta_syn[:, 1:2], 0.0)
    tt_e = t_e / Tau_syn[:, 0:1] ** 2
    tt_i = t_i / Tau_syn[:, 1:2] ** 2
    e_kern = tt_e * np.exp(-tt_e) * W_syn[:, 0:1] ** 2
    i_kern = -(tt_i * np.exp(-tt_i)) * W_syn[:, 1:2] ** 2
    ekm = np.ascontiguousarray(e_kern[:, ::-1].T).astype(BF16)  # [T_NO, SUB]
    ikm = np.ascontiguousarray(i_kern[:, ::-1].T).astype(BF16)

    def pack_rows(mat, nch):
        # [R, C] -> [128, nch*C]: chunk kc rows at columns [C*kc, C*(kc+1))
        r, c = mat.shape
        out = np.zeros((128, nch * c), np.float32)
        for kc in range(nch):
            rows = mat[128 * kc : min(r, 128 * (kc + 1))]
            out[: rows.shape[0], c * kc : c * kc + c] = rows
        return out

    w1t = np.asarray(W1, np.float32).T
    w2t = np.asarray(W2, np.float32).T
    w3t = np.asarray(W3, np.float32).T
    w4t = np.asarray(W4, np.float32).T
    n_kw, n_kh, n_ks = _ceil_div(WIN, 128), _ceil_div(HID, 128), _ceil_div(T_NO, 128)
    spk = np.concatenate(
        [pack_rows(w4t, n_kh), pack_rows(ekm.astype(np.float32), n_ks),
         pack_rows(ikm.astype(np.float32), n_ks)], 1)
    bpk = np.concatenate(
        [pack_rows(np.asarray(b, np.float32)[:, None], n_kh).reshape(128, n_kh)
         for b in (b1, b2, b3)], 1)
    wd = {
        "w1t": pack_rows(w1t, n_kw).astype(BF16),
        "w2t": pack_rows(w2t, n_kh).astype(BF16),
        "w3t": pack_rows(w3t, n_kh).astype(BF16),
        "bpk": np.ascontiguousarray(bpk, np.float32),
        "b4": np.asarray(b4, np.float32).astype(BF16),
        "spk": spk.astype(BF16),
    }

    vg = np.zeros(T_NO - 1 + T + WIN + 128 + T_PAD - T_LOC, np.float32)
    vg[T_NO - 1 : T_NO - 1 + T] = V
    vg = vg.astype(BF16)

    halo = T_NO - 1
    ez = np.zeros((halo, S_e.shape[1]), np.float32)
    iz = np.zeros((halo, S_i.shape[1]), np.float32)
    in_maps = []
    for m in range(N_CORES):
        r0 = m * T_LOC
        if m == 0:
            se_m = np.concatenate([ez, S_e[:T_LOC]], 0)
            si_m = np.concatenate([iz, S_i[:T_LOC]], 0)
        else:
            se_m = S_e[r0 - halo : r0 + T_LOC]
            si_m = S_i[r0 - halo : r0 + T_LOC]
        in_maps.append(
            {"se": se_m, "si": si_m, "v": vg[r0 : r0 + V_LEN], **wd}
        )

    nc = _build(T_PAD, SE_ROWS, S_e.shape[1], S_i.shape[1], HID, SUB)
    trace = os.environ.get("CC_TRACE") == "1"
    res = run_bass_kernel_spmd(nc, in_maps, list(range(N_CORES)), trace=trace)
    LAST["exec_time_ns"] = res.exec_time_ns
    LAST["results"] = res
    out = np.concatenate(
        [res.results[m]["out"][:, :T_LOC].T for m in range(N_CORES)], 0
    )
    return np.ascontiguousarray(out.astype(np.float32))



# revision 17
# speedup vs baseline: 1.4713x; 1.4713x over previous
"""Trainium2 Bass kernel for nn_Encoder (sliding-window MLP + synaptic conv).

Computation (per timestep t of T_data):
  syn_e[t] = sum(S_e[t, :]);  syn_i[t] = sum(S_i[t, :])
  syn_out[t, s] = sum_k e_kern[s, k] * syn_e[t-k] + i_kern[s, k] * syn_i[t-k]
  Vw[t, :] = V[t-199 : t+200]   (zero padded)
  h = lrelu(Vw @ W1.T + b1); h = lrelu(h @ W2.T + b2); h = lrelu(h @ W3.T + b3)
  out[t, :] = tanh(h @ W4.T + b4 + syn_out[t, :])

Strategy: data-parallel over T across 8 NeuronCores (T/8 slice + 199-halo
per core).  On each core:
  - S_e/S_i are uploaded TRANSPOSED in fp8-e4m3 (row-sum washes out the
    quantization; fp8 quarters the dominant HBM stream).  Row-sums run on
    the PE as ones-stationary DoubleRow matmuls (VectorE reduce has no
    fast uop and would cost ~110us); sums are evacuated in bf16 to a DRAM
    scratch and re-read as Hankel conv operands.
  - The MLP runs in fp8 DoubleRow (2 contraction rows per pass): weights
    are K-padded to 512 and packed [128, 4, 512]; the V window streams
    straight out of the Hankel SBUF tile via an overlapping 3-D AP.
  - Biases ride inside the stationaries: h carries a constant-1 row at
    hid-index 500 (created by L1's evacuation bias), and W2/W3/W4 row 500
    holds b2/b3/b4 (plus a 1.0 diagonal to regenerate the ones row).
  - L4 and the conv accumulate into the SAME [20, nt] PSUM slot (conv
    matmuls join two pairs later, once the sum scratch exists); slots are
    packed 4-per-bank at partitions 0/32/64/96.  A single Tanh evacuates.
  - PSUM->SBUF lrelu evacuations are split between ScalarE (Activation
    with bias) and VectorE (single fused scalar_tensor_tensor max(x,.01x)).
"""

import os
from contextlib import ExitStack

import ml_dtypes
import numpy as np

import concourse.bass as bass
import concourse.mybir as mybir
import concourse.tile as tile
from concourse import bacc
from concourse.bass_utils import run_bass_kernel_spmd
from concourse.tile_rust import add_dep_helper

FP8 = ml_dtypes.float8_e4m3fn
BF16 = ml_dtypes.bfloat16
FP32 = mybir.dt.float32
BF = mybir.dt.bfloat16
F8 = mybir.dt.float8e4
DR = mybir.MatmulPerfMode.DoubleRow

T_NO = 200
WIN = 2 * T_NO - 1  # 399
N_CORES = 8
BLK = 512
HIDP = 512  # hid (500) padded; row 500 = constant-1 / bias row
WINP = 512  # window (399) padded
SUBP = 32   # sub (20) padded in the W4 stationary free dim
VH_W = 384  # extra hankel cols: DR pass1 reads cols up to 256+128+nt-1
SY_W = 128  # conv hankel extra cols (chunks at 0 and 128)

LAST = {}


def _ceil_div(a, b):
    return -(-a // b)


def _build(T_PAD, L_PAD, SUB):
    NB = _ceil_div(T_PAD, BLK)
    SE_G = 8  # 8 tiles of 256 transposed S_e rows (2048 pad)
    SI_G = 2  # 2 tiles of 256 transposed S_i rows (512 pad)
    V_LEN = T_PAD + WIN - 1 + 128
    RB = L_PAD // BLK  # rowsum blocks over the scratch domain

    nc = bacc.Bacc(
        "TRN2", target_bir_lowering=False, debug=False, num_devices=N_CORES
    )

    set_h = nc.dram_tensor("set", [SE_G * 256, L_PAD], F8, kind="ExternalInput")
    sit_h = nc.dram_tensor("sit", [SI_G * 256, L_PAD], F8, kind="ExternalInput")
    v_h = nc.dram_tensor("v", [V_LEN], F8, kind="ExternalInput")
    cp8_h = nc.dram_tensor("cp8", [128, 3 * 4 * HIDP + 4 * SUBP + 2 * 32], F8,
                           kind="ExternalInput")
    cp16_h = nc.dram_tensor("cp16", [128, 4 * SUBP], BF, kind="ExternalInput")
    cpf_h = nc.dram_tensor("cpf", [128, 4], FP32, kind="ExternalInput")
    out_h = nc.dram_tensor("out", [SUB, T_PAD], FP32, kind="ExternalOutput")
    scr_h = nc.dram_tensor("scr", [2, L_PAD], BF)

    with tile.TileContext(nc) as tc, ExitStack() as ctx:
        cpool = ctx.enter_context(tc.tile_pool(name="consts", bufs=1))
        spool = ctx.enter_context(tc.tile_pool(name="sdata", bufs=1))
        tmppool = ctx.enter_context(tc.tile_pool(name="evtmp", bufs=3))
        hkpool = ctx.enter_context(tc.tile_pool(name="hankel", bufs=3))
        sypool = ctx.enter_context(tc.tile_pool(name="synh", bufs=3))
        hpool = ctx.enter_context(tc.tile_pool(name="acts", bufs=2))
        smpool = ctx.enter_context(tc.tile_pool(name="sums", bufs=3))
        opool = ctx.enter_context(tc.tile_pool(name="outs", bufs=3))
        psmm = ctx.enter_context(tc.tile_pool(name="psmm", bufs=5, space="PSUM"))
        ps4p = ctx.enter_context(tc.tile_pool(name="ps4p", bufs=1, space="PSUM"))
        psrs = ctx.enter_context(tc.tile_pool(name="psrs", bufs=1, space="PSUM"))

        # ---- constants ----
        def cload(nm, shape, dram, off, width):
            t = cpool.tile(shape, dram.dtype, name=nm, tag=nm)
            ap_dims = [[dram.shape[1], 128]]
            rem = shape[1:]
            if len(rem) == 2:
                ap_dims += [[rem[1], rem[0]], [1, rem[1]]]
            else:
                ap_dims += [[1, rem[0]]]
            d = nc.sync.dma_start(
                out=t[tuple([slice(None)] * len(shape))],
                in_=bass.AP(dram, off, ap_dims),
            )
            return t, d

        w1t, w_dma = cload("w1t", [128, 4, HIDP], cp8_h, 0, 4 * HIDP)
        w2t, _ = cload("w2t", [128, 4, HIDP], cp8_h, 4 * HIDP, 4 * HIDP)
        w3t, _ = cload("w3t", [128, 4, HIDP], cp8_h, 8 * HIDP, 4 * HIDP)
        w4t, _ = cload("w4t", [128, 4, SUBP], cp8_h, 12 * HIDP, 4 * SUBP)
        seo, _ = cload("seo", [128, 2, 16], cp8_h, 12 * HIDP + 4 * SUBP, 32)
        sio, _ = cload("sio", [128, 2, 16], cp8_h, 12 * HIDP + 4 * SUBP + 32, 32)
        kpk, _ = cload("kpk", [128, 4, SUBP], cp16_h, 0, 4 * SUBP)
        bias1, _ = cload("bias1", [128, 4], cpf_h, 0, 4)
        c001 = cpool.tile([128, 1], FP32, name="c001", tag="c001")
        nc.vector.memset(c001[:, :], 0.01)

        # ---- resident transposed S tiles; loaded in column segments ----
        se_sb = [spool.tile([128, 2, L_PAD], F8, name=f"se{g}", tag=f"se{g}")
                 for g in range(SE_G)]
        si_sb = [spool.tile([128, 2, L_PAD], F8, name=f"si{g}", tag=f"si{g}")
                 for g in range(SI_G)]
        SEG = 2 * BLK  # one segment feeds two rowsum blocks
        N_SEG = _ceil_div(L_PAD, SEG)

        def emit_seg(s, gate=None):
            c0 = SEG * s
            c1 = min(L_PAD, c0 + SEG)
            first = None
            for tiles, dram in ((se_sb, set_h), (si_sb, sit_h)):
                for g, t in enumerate(tiles):
                    d = nc.sync.dma_start(
                        out=t[:, :, c0:c1],
                        in_=bass.AP(
                            dram, 256 * g * L_PAD + c0,
                            [[L_PAD, 128], [128 * L_PAD, 2], [1, c1 - c0]],
                        ),
                    )
                    if first is None:
                        first = d
            if gate is not None:
                add_dep_helper(first.ins, gate.ins, sync=True,
                               reason="startup loads first")
            return first

        # ---- rowsum block rb -> scratch cols [BLK*rb, BLK*rb+BLK) ----
        def emit_rowsum(rb):
            c0 = BLK * rb
            nt = min(BLK, L_PAD - c0)
            ps = psrs.tile([16, BLK], FP32, name="psrs", tag="psrs")
            for g in range(SE_G):
                nc.tensor.matmul(
                    ps[:, :nt], seo[:, :, :], se_sb[g][:, :, c0:c0 + nt],
                    start=(g == 0), stop=False, perf_mode=DR,
                )
            for g in range(SI_G):
                nc.tensor.matmul(
                    ps[:, :nt], sio[:, :, :], si_sb[g][:, :, c0:c0 + nt],
                    start=False, stop=(g == SI_G - 1), perf_mode=DR,
                )
            sm = smpool.tile([2, BLK], BF, name="sums", tag="sums")
            nc.vector.tensor_copy(sm[:, :nt], ps[0:2, :nt])
            nc.scalar.dma_start(
                out=bass.AP(scr_h, c0, [[L_PAD, 2], [1, nt]]), in_=sm[:, :nt]
            )

        # ---- pairs of timestep blocks ----
        pairs = [tuple(b for b in (i, i + 1) if b < NB) for i in range(0, NB, 2)]
        NP = len(pairs)
        pair_blks = []
        for pair in pairs:
            blks = []
            off = 0
            for b in pair:
                nt = min(BLK, T_PAD - BLK * b)
                blks.append((BLK * b, nt, off))
                off += nt
            pair_blks.append((blks, off))

        synh_tiles = {}
        ps4_tiles = {}  # global block idx -> (psum tile, base partition)
        ps4_pool_tiles = {}

        def ps4_slot(b):
            # 6 live [20, nt] slots packed 3-per-bank at partitions 0/32/64
            # (base partition 96 is not supported by bass AP lowering)
            ti, sl = (b // 3) % 2, b % 3
            key = (b // 3, ti)
            if key not in ps4_pool_tiles:
                ps4_pool_tiles[key] = ps4p.tile([128, BLK], FP32, name="ps4",
                                                tag=f"ps4_{ti}")
            return ps4_pool_tiles[key], 32 * sl

        def emit_synh(pi):
            t0p = BLK * pairs[pi][0]
            tot = pair_blks[pi][1]
            synh = {}
            for row, nm in ((0, "se"), (1, "si")):
                t = sypool.tile([128, 2 * BLK + SY_W], BF, name=f"{nm}h",
                                tag=f"{nm}h")
                nc.sync.dma_start(
                    out=t[:, : tot + SY_W],
                    in_=bass.AP(scr_h, row * L_PAD + t0p,
                                [[1, 128], [1, tot + SY_W]]),
                )
                synh[nm] = t
            synh_tiles[pi] = synh

        def emit_conv(pi):
            blks, tot = pair_blks[pi]
            synh = synh_tiles.pop(pi)
            for bi, (bt0, nt, coff) in enumerate(blks):
                b = bt0 // BLK
                ps4, bp = ps4_tiles.pop(b)
                for j, (nm, ko) in enumerate(
                    (("se", 0), ("se", 1), ("si", 2), ("si", 3))
                ):
                    pk = 128 if ko % 2 == 0 else T_NO - 128
                    nc.tensor.matmul(
                        ps4[bp:bp + SUB, :nt], kpk[:pk, ko, :SUB],
                        synh[nm][:pk, coff + 128 * (ko % 2):
                                 coff + 128 * (ko % 2) + nt],
                        start=False, stop=(j == 3), skip_group_check=True,
                    )
                out_sb = opool.tile([SUB, BLK], FP32, name="out_sb", tag="out_sb")
                nc.scalar.activation(out_sb[:, :nt], ps4[bp:bp + SUB, :nt],
                                     mybir.ActivationFunctionType.Tanh)
                nc.scalar.dma_start(out=out_h[:, bt0:bt0 + nt], in_=out_sb[:, :nt])

        # evacuation engine per (layer idx 0..2, m-chunk 0..3):
        # L1 needs the bias (ones-row creation) -> Activation only.
        # "V": DVE copies PSUM->SBUF bf16, then applies lrelu SBUF->SBUF
        # (a single DVE op cannot read two PSUM operands; GpSimd has no
        # PSUM port and no TensorScalarPtr opcode).
        EVAC = {
            (0, 0): "A", (0, 1): "A", (0, 2): "A", (0, 3): "A",
            (1, 0): "V", (1, 1): "V", (1, 2): "V", (1, 3): "V",
            (2, 0): "A", (2, 1): "A", (2, 2): "A", (2, 3): "A",
        }

        def emit_evac(lidx, mc, h_t, ps, nt):
            dst = h_t[:, mc, :nt]
            if EVAC[(lidx, mc)] == "A":
                nc.scalar.activation(
                    dst, ps[:, :nt], mybir.ActivationFunctionType.Lrelu,
                    bias=bias1[:, mc:mc + 1] if lidx == 0 else 0.0,
                    alpha=0.01,
                )
            else:
                tmp = tmppool.tile([128, BLK], BF, name="evtmp", tag="evtmp")
                nc.vector.tensor_copy(tmp[:, :nt], ps[:, :nt])
                nc.vector.scalar_tensor_tensor(
                    dst, tmp[:, :nt], c001[:, 0:1], tmp[:, :nt],
                    mybir.AluOpType.mult, mybir.AluOpType.max,
                )

        rb_next = 0

        def emit_rowsums_until(tgt):
            nonlocal rb_next
            while rb_next < min(tgt, RB):
                emit_rowsum(rb_next)
                rb_next += 1

        for pi, pair in enumerate(pairs):
            blks, tot = pair_blks[pi]
            t0p = BLK * pair[0]

            vh = hkpool.tile([128, 2 * BLK + VH_W], F8, name="vh", tag="vh")
            vh_dma = nc.sync.dma_start(
                out=vh[:, : tot + VH_W],
                in_=bass.AP(v_h, t0p, [[1, 128], [1, tot + VH_W]]),
            )
            if pi == 0:
                emit_seg(0, gate=vh_dma)
                emit_seg(1)
            if pi + 2 < N_SEG:
                emit_seg(pi + 2)

            # layers 1..3, fp8 DoubleRow, block-pair interleaved; the
            # PE-side rowsum matmuls slot in after L2 (their S-segment
            # loads lead by two pairs, so they never head-of-line block)
            h_prev = {}
            for lidx, w_t in enumerate((w1t, w2t, w3t)):
                for bi, (bt0, nt, coff) in enumerate(blks):
                    h_t = hpool.tile([128, 4, BLK], F8, name=f"h{lidx}",
                                     tag=f"h{lidx}_{bi}")
                    for mc in range(4):
                        ps = psmm.tile([128, BLK], FP32, name="ps", tag="ps")
                        for P in range(2):
                            if lidx == 0:
                                vb = vh[:, :]
                                rhs = bass.AP(
                                    vb.tensor, vb.offset + coff + 256 * P,
                                    [[2 * BLK + VH_W, 128], [128, 2], [1, nt]],
                                )
                            else:
                                rhs = h_prev[bi][:, 2 * P:2 * P + 2, :nt]
                            nc.tensor.matmul(
                                ps[:, :nt],
                                w_t[:, 2 * P:2 * P + 2,
                                    128 * mc:128 * (mc + 1)],
                                rhs,
                                start=(P == 0), stop=(P == 1), perf_mode=DR,
                            )
                        emit_evac(lidx, mc, h_t, ps, nt)
                    h_prev[bi] = h_t
                if lidx == 1:
                    emit_rowsums_until(2 * pi + 2)
                elif lidx == 2:
                    emit_rowsums_until(2 * pi + 4)

            # layer 4 feed-forward into the shared ps4 slot (conv joins later);
            # plain fp8 matmuls: DoubleRow requires dst partition 0, but the
            # packed slots sit at partitions 0/32/64
            for bi, (bt0, nt, coff) in enumerate(blks):
                b = bt0 // BLK
                ps4, bp = ps4_slot(b)
                ps4_tiles[b] = (ps4, bp)
                for kc in range(4):
                    nc.tensor.matmul(
                        ps4[bp:bp + SUB, :nt],
                        w4t[:, kc, :SUB],
                        h_prev[bi][:, kc, :nt],
                        start=(kc == 0), stop=False,
                        skip_group_check=True,
                    )

            if pi >= 1:
                emit_synh(pi - 1)
            if pi >= 2:
                emit_conv(pi - 2)

        emit_rowsums_until(RB)
        emit_synh(NP - 1)
        for pi in range(max(0, NP - 2), NP):
            emit_conv(pi)

    nc.compile()
    return nc


def kernel(V, S_e, S_i, W1, b1, W2, b2, W3, b3, W4, b4, W_syn, Tau_syn, Delta_syn):
    V = np.asarray(V, np.float32)
    T = V.shape[0]
    assert T % N_CORES == 0
    T_LOC = T // N_CORES
    T_PAD = _ceil_div(T_LOC, 128) * 128
    halo = T_NO - 1
    L_PAD = _ceil_div(T_LOC + halo, BLK) * BLK  # transposed-S / scratch cols
    V_LEN = T_PAD + WIN - 1 + 128
    HID = W1.shape[0]
    SUB = W4.shape[0]

    # ---- synaptic kernels (tiny, host fp32) ----
    W_syn = np.asarray(W_syn, np.float32)
    Tau_syn = np.asarray(Tau_syn, np.float32)
    Delta_syn = np.asarray(Delta_syn, np.float32)
    t_raw = np.arange(T_NO, dtype=np.float32)[None, :]
    tt_e = np.maximum(t_raw - Delta_syn[:, 0:1], 0.0) / Tau_syn[:, 0:1] ** 2
    tt_i = np.maximum(t_raw - Delta_syn[:, 1:2], 0.0) / Tau_syn[:, 1:2] ** 2
    e_kern = tt_e * np.exp(-tt_e) * W_syn[:, 0:1] ** 2
    i_kern = -(tt_i * np.exp(-tt_i)) * W_syn[:, 1:2] ** 2
    ekm = np.ascontiguousarray(e_kern[:, ::-1].T)  # [T_NO, SUB]
    ikm = np.ascontiguousarray(i_kern[:, ::-1].T)
    kpk = np.zeros((128, 4, SUBP), np.float32)
    kpk[:, 0, :SUB] = ekm[:128]
    kpk[:T_NO - 128, 1, :SUB] = ekm[128:]
    kpk[:, 2, :SUB] = ikm[:128]
    kpk[:T_NO - 128, 3, :SUB] = ikm[128:]

    # ---- DoubleRow weight packs: [128, 4, M], row 500 carries bias/ones ----
    def dr3(mat_pad):
        k, m = mat_pad.shape
        return np.ascontiguousarray(
            mat_pad.reshape(4, 128, m).transpose(1, 0, 2)
        )

    w1p = np.zeros((WINP, HIDP), np.float32)
    w1p[:WIN, :HID] = np.asarray(W1, np.float32).T
    w2p = np.zeros((HIDP, HIDP), np.float32)
    w2p[:HID, :HID] = np.asarray(W2, np.float32).T
    w2p[HID, :HID] = np.asarray(b2, np.float32)
    w2p[HID, HID] = 1.0
    w3p = np.zeros((HIDP, HIDP), np.float32)
    w3p[:HID, :HID] = np.asarray(W3, np.float32).T
    w3p[HID, :HID] = np.asarray(b3, np.float32)
    w3p[HID, HID] = 1.0
    w4p = np.zeros((HIDP, SUBP), np.float32)
    w4p[:HID, :SUB] = np.asarray(W4, np.float32).T
    w4p[HID, :SUB] = np.asarray(b4, np.float32)

    seo = np.zeros((128, 2, 16), np.float32)
    seo[:, :, 0] = 1.0
    sio = np.zeros((128, 2, 16), np.float32)
    sio[:, :, 1] = 1.0
    cp8 = np.concatenate(
        [dr3(w1p).reshape(128, -1), dr3(w2p).reshape(128, -1),
         dr3(w3p).reshape(128, -1), dr3(w4p).reshape(128, -1),
         seo.reshape(128, -1), sio.reshape(128, -1)], axis=1
    ).astype(FP8)

    bias1 = np.zeros((128, 4), np.float32)
    b1f = np.asarray(b1, np.float32)
    for mc in range(4):
        rows = b1f[128 * mc: min(HID, 128 * (mc + 1))]
        bias1[: rows.shape[0], mc] = rows
    bias1[HID - 384, 3] = 1.0  # ones-row seed in h1

    # ---- transposed fp8 S uploads (padded [2048|512, L_PAD]) ----
    S_e8 = np.asarray(S_e, np.float32).astype(FP8)
    S_i8 = np.asarray(S_i, np.float32).astype(FP8)
    SeT = np.ascontiguousarray(S_e8.T)  # [2000, T]
    SiT = np.ascontiguousarray(S_i8.T)  # [500, T]

    vg = np.zeros(halo + T + WIN + 128 + T_PAD - T_LOC, np.float32)
    vg[halo: halo + T] = V
    vg = vg.astype(FP8)

    wd = {
        "cp8": cp8,
        "cp16": np.ascontiguousarray(kpk.reshape(128, -1)).astype(BF16),
        "cpf": np.ascontiguousarray(bias1),
    }
    in_maps = []
    for m in range(N_CORES):
        r0 = m * T_LOC
        set_m = np.zeros((2048, L_PAD), FP8)
        sit_m = np.zeros((512, L_PAD), FP8)
        if m == 0:
            set_m[:SeT.shape[0], halo: halo + T_LOC] = SeT[:, :T_LOC]
            sit_m[:SiT.shape[0], halo: halo + T_LOC] = SiT[:, :T_LOC]
        else:
            set_m[:SeT.shape[0], : halo + T_LOC] = SeT[:, r0 - halo: r0 + T_LOC]
            sit_m[:SiT.shape[0], : halo + T_LOC] = SiT[:, r0 - halo: r0 + T_LOC]
        in_maps.append(
            {"set": set_m, "sit": sit_m, "v": vg[r0: r0 + V_LEN], **wd}
        )

    nc = _build(T_PAD, L_PAD, SUB)
    trace = os.environ.get("CC_TRACE") == "1"
    res = run_bass_kernel_spmd(nc, in_maps, list(range(N_CORES)), trace=trace)
    LAST["exec_time_ns"] = res.exec_time_ns
    LAST["results"] = res
    out = np.concatenate(
        [res.results[m]["out"][:, :T_LOC].T for m in range(N_CORES)], 0
    )
    return np.ascontiguousarray(out.astype(np.float32))
